# revision 14
# baseline (speedup 1.0000x reference)
"""Trainium2 Bass kernel for nn_DiffModel_53764400611855.

The 160000-point stream collapses algebraically to per-segment coordinate
sums u[s] (segment_sum and quat rotation are linear in the points), and the
batchnorm layers cancel every bias that is constant across the 640-segment
batch (pe_b, pfc_b, o_b1, o_b2).  What remains is:

  h1T = (W_all @ o_w1)^T @ X_all          with
  W_all rows / X_all rows:
     pfcA   (128) <->  nerfA  = sin(2pi * reduce(GA' x + bA'))   [128,640]
     pfcBs   (12) <->  nerfBs = sin(...)                          [12,640]
     pfcBi+pe_w(7)<->  xT     = noise_param^T                      [7,640]
     pe_w     (3) <->  uT     = per-seg point sums / 250           [3,640]
     2*pe_w/250(3)<->  mT     = (w*(v x u) + v x (v x u)) / |q|^2  [3,640]
     temb2   (32) <->  Bsel   = kron(I32, 1_20)                   [32,640]
  then bn+relu -> @o_w2 -> bn+relu -> @o_w3 + b3.

All matmuls run in bf16 (fp32 PSUM accumulate) except the trig-argument
matmuls, which stay fp32 for phase accuracy.  sin() uses a 3-op range
reduction (f32->i32 cast rounds to nearest on this HW) + one ACT Sin with
scale=2pi.  BatchNorm moments come from bn_stats/bn_aggr; the scale, shift,
relu and bf16 cast fuse into one ACT per tile.  Only two ACT table sets are
used (silu_and_others, sqrt_and_others).

All 8 cores run the same replicated program (no collectives); core 0's
output is returned.  Hardcodes the fixed input structure: contiguous
segments of 250 points, batch_length == 250.
"""

import numpy as np
import ml_dtypes

NCORES = 8
S, C, PPP, BO = 640, 512, 250, 32
NJ = S // 128               # seg-major blocks = 5
PI = float(np.pi)
TWO_PI = float(2.0 * np.pi)
INV2PI = float(1.0 / (2.0 * np.pi))

_CACHE = {}


def _consts():
    f = np.float32
    # nerf A block: sc-flat cols 0..127 (bands 0..9 partial), with /2pi
    # prescale and bias row (0.25 turn for cos entries)
    GAs = np.zeros((8, 128), f)
    for i in range(128):
        fb, k = i // 14, i % 14
        GAs[k % 7, i] = (2.0 ** fb) * INV2PI
        GAs[7, i] = 0.25 if k >= 7 else 0.0
    # B block: sc-flat cols 128..139 (band 9, k=2..13)
    GBs = np.zeros((8, 12), f)
    for j in range(12):
        k = 2 + j
        GBs[k % 7, j] = (2.0 ** 9) * INV2PI
        GBs[7, j] = 0.25 if k >= 7 else 0.0
    freqs = np.exp(
        -np.log(10000.0) * np.arange(256, dtype=f) / 256.0
    ).astype(f)
    fq = np.zeros((2, 256), f)
    fq[0] = freqs * INV2PI
    fq[1] = 0.25
    Bsel = np.kron(np.eye(BO, dtype=f), np.ones((1, 20), f))
    return GAs, GBs, fq, np.ascontiguousarray(Bsel)


def _build_nc():
    import concourse.mybir as mybir
    import concourse.tile as tile
    from concourse import bacc, masks

    f32, i32, bf16 = mybir.dt.float32, mybir.dt.int32, mybir.dt.bfloat16
    AF = mybir.ActivationFunctionType
    ALU = mybir.AluOpType
    AX = mybir.AxisListType

    nc = bacc.Bacc(None, num_devices=NCORES)

    def din(name, shape, dt=f32):
        return nc.dram_tensor(name, shape, dt, kind="ExternalInput")

    # consolidated input blobs (few big DMAs; see _in_maps for layouts)
    d_f32A = din("f32A", [8, 1068])
    d_f32B = din("f32B", [128, 43])
    d_ts = din("ts", [1, BO], i32)
    d_bfS = din("bfS", [1, 1024], bf16)
    d_W1 = din("W1", [128, 3824], bf16)
    d_W2 = din("W2", [128, 2311], bf16)
    d_pc = din("pc", [S, PPP * 3], bf16)
    d_xTb = din("xTb", [7, S], bf16)
    d_Bsel = din("Bsel", [BO, S], bf16)
    d_out = nc.dram_tensor("outT", [7, S], f32, kind="ExternalOutput")

    with tile.TileContext(nc) as tc:
        with (
            tc.tile_pool(name="const", bufs=1) as cp,
            tc.tile_pool(name="work", bufs=1) as wp,
            tc.tile_pool(name="ps_pre", bufs=2, space="PSUM") as pp_pre,
            tc.tile_pool(name="ps_mlp", bufs=1, space="PSUM") as pp_mlp,
            tc.tile_pool(name="ps_trp", bufs=1, space="PSUM") as pp_trp,
            tc.tile_pool(name="ps_head", bufs=4, space="PSUM") as pp_head,
        ):
            # ---------------- DMAs ----------------
            # sync ring: small f32 first, then pc j0-1, then weight blobs
            f32A = cp.tile([8, 1068], f32, tag="f32A")
            nc.sync.dma_start(f32A[:], d_f32A[:])
            ts_i = cp.tile([1, BO], i32, tag="ts_i")
            nc.sync.dma_start(ts_i[:], d_ts[:])
            f32B = cp.tile([128, 43], f32, tag="f32B")
            nc.sync.dma_start(f32B[:], d_f32B[:])
            bfS = cp.tile([1, 1024], bf16, tag="bfS")
            nc.sync.dma_start(bfS[:], d_bfS[:])
            pcb = wp.tile([128, NJ, PPP * 3], bf16, tag="pcb")
            pc_r = d_pc.rearrange("(j p) k -> p j k", p=128)
            nc.sync.dma_start(pcb[:, 0:2, :], pc_r[:, 0:2, :])
            W1 = cp.tile([128, 3824], bf16, tag="W1")
            nc.sync.dma_start(W1[:], d_W1[:])
            W2 = cp.tile([128, 2311], bf16, tag="W2")
            nc.sync.dma_start(W2[:], d_W2[:])
            # scalar ring (2nd HWDGE): pc j2-4 + X-side inputs
            nc.scalar.dma_start(pcb[:, 2:5, :], pc_r[:, 2:5, :])
            X1a = wp.tile([19, S], bf16, tag="X1a")
            nc.scalar.dma_start(X1a[12:19, :], d_xTb[:])
            X1b = wp.tile([38, S], bf16, tag="X1b")
            nc.scalar.dma_start(X1b[6:38, :], d_Bsel[:])

            # views into the blobs
            xTf = f32A[:, 0:640]
            GAs = f32A[:, 640:768]
            GBs = f32A[:, 768:780]
            fq = f32A[0:2, 780:1036]
            npseg = f32B[:, 0:35]
            bn1g = f32B[:, 35:37]
            bn1b = f32B[:, 37:39]
            bn2g = f32B[:, 39:40]
            bn2b = f32B[:, 40:41]
            ob3c = f32B[0:7, 41:42]
            tb1r = bfS[0:1, 0:512]
            tb2r = bfS[0:1, 512:1024]
            tw1p = [W1[:, 512 * k:512 * (k + 1)] for k in range(4)]
            pfcAT = W1[:, 2048:2560]
            ow1 = [W1[:, 2560 + 256 * k:2560 + 256 * (k + 1)]
                   for k in range(4)]
            Wa = W1[:, 3584:3660].rearrange("p (k r) -> p k r", r=19)
            Wb = W1[:, 3660:3812].rearrange("p (k r) -> p k r", r=38)
            pewT = W1[:, 3812:3824].rearrange("p (k r) -> p k r", r=3)
            tw2 = [W2[:, 512 * k:512 * (k + 1)] for k in range(4)]
            ow2c = W2[:, 2048:2304].rearrange("p (k n) -> p k n", n=128)
            ow3 = W2[:, 2304:2311]

            ident = cp.tile([128, 128], f32, tag="ident")
            masks.make_identity(nc, ident[:])
            ones1 = cp.tile([1, BO], bf16, tag="ones1")
            nc.gpsimd.memset(ones1[:], 1.0)
            dum = cp.tile([1, 1], f32, tag="dum")
            nc.gpsimd.memset(dum[:], 1.0)
            dum2 = cp.tile([1, 1], f32, tag="dum2")
            dum3 = cp.tile([1, 1], f32, tag="dum3")
            eps128 = cp.tile([128, 1], f32, tag="eps128")
            nc.gpsimd.memset(eps128[:], 1e-5)
            # first ACT op -> loads silu_and_others (sin+silu+copy+relu)
            nc.scalar.activation(dum2[:], dum[:], AF.Silu)

            tm2 = wp.tile([2, BO], f32, tag="tm2")
            nc.vector.tensor_copy(tm2[0:1, :], ts_i[:])
            nc.sync.dma_start(tm2[1:2, :], d_f32A[0:1, 1036:1068])

            # ---------------- points reduce (DVE, arrival order) --------
            # q6 cols per j: u(3) m(3);  u filled by reduces, m by quat
            q6 = wp.tile([128, NJ * 6], f32, tag="q6")
            for j in (2, 3, 4, 0, 1):
                nc.vector.tensor_reduce(
                    q6[:, 6 * j:6 * j + 3],
                    pcb[:, j, :].rearrange("p (c k) -> p c k", c=3),
                    axis=AX.X, op=ALU.add,
                )

            # ---------------- trig helper ----------------
            def sin_tile(ps_ap, P, W, tag, dst):
                # dst = sin(2pi * frac(ps)); casts on gpsimd, sub on DVE
                ti_ = wp.tile([P, W], i32, tag=f"{tag}i", name=f"{tag}i")
                tf_ = wp.tile([P, W], f32, tag=f"{tag}f", name=f"{tag}f")
                rr_ = wp.tile([P, W], f32, tag=f"{tag}r", name=f"{tag}r")
                nc.vector.tensor_copy(ti_[:], ps_ap)
                nc.gpsimd.tensor_copy(tf_[:], ti_[:])
                nc.vector.tensor_sub(rr_[:], ps_ap, tf_[:])
                nc.scalar.activation(dst, rr_[:], AF.Sin, scale=TWO_PI)

            # argt: [cos0 | sin0 | cos1 | sin1] blocks of 32 cols
            argt = pp_trp.tile([128, 128], f32, tag="trp", name="argt")
            for r in range(2):
                fsl = slice(128 * r, 128 * (r + 1))
                nc.tensor.matmul(
                    argt[:, 64 * r:64 * r + 32], fq[:, fsl], tm2[:],
                    start=True, stop=True,
                )
                nc.tensor.matmul(
                    argt[:, 64 * r + 32:64 * r + 64], fq[0:1, fsl],
                    tm2[0:1, :], start=True, stop=True,
                )
            embT = wp.tile([128, 128], bf16, tag="embT")
            sin_tile(argt[:], 128, 128, "at", embT[:])

            # nerf args (fp32 matmuls for phase accuracy)
            X0 = wp.tile([128, S], bf16, tag="X0")
            for h in range(2):
                sl = slice(320 * h, 320 * (h + 1))
                psA = pp_pre.tile([128, 320], f32, tag="pre", name="psA")
                nc.tensor.matmul(
                    psA[:], GAs, xTf[:, sl], start=True, stop=True
                )
                sin_tile(psA[:], 128, 320, f"nA{h}", X0[:, sl])
            for h in range(2):
                sl = slice(320 * h, 320 * (h + 1))
                psB = pp_pre.tile([128, 320], f32, tag="pre", name="psB")
                nc.tensor.matmul(
                    psB[0:12, :], GBs, xTf[:, sl], start=True, stop=True
                )
                sin_tile(psB[0:12, :], 12, 320, f"nB{h}", X1a[0:12, sl])

            # ---------------- quaternions (comp-major packed) -----------
            # npsegv [p, c(7), j(5)]; quat = comps 3..6
            npsegv = npseg.rearrange("p (j c) -> p c j", c=7)
            q6v = q6[:, :].rearrange("p (j c) -> p c j", c=6)
            sq = wp.tile([128, NJ * 4], f32, tag="sq")
            sq_v = sq[:, :].rearrange("p (j c) -> p j c", c=4)
            nc.vector.tensor_mul(
                sq_v, npseg.rearrange("p (j c) -> p j c", c=7)[:, :, 3:7],
                npseg.rearrange("p (j c) -> p j c", c=7)[:, :, 3:7],
            )
            n2 = wp.tile([128, NJ], f32, tag="n2")
            nc.vector.tensor_reduce(n2[:], sq_v, axis=AX.X, op=ALU.add)
            rn2 = wp.tile([128, NJ], f32, tag="rn2")
            nc.vector.reciprocal(rn2[:], n2[:])
            # duplicated comp-major tiles (copies on gpsimd)
            v5 = wp.tile([128, 5, NJ], f32, tag="v5")
            nc.gpsimd.tensor_copy(v5[:, 0:3, :], npsegv[:, 4:7, :])
            nc.gpsimd.tensor_copy(v5[:, 3:5, :], npsegv[:, 4:6, :])
            u5 = wp.tile([128, 5, NJ], f32, tag="u5")
            nc.gpsimd.tensor_copy(u5[:, 0:3, :], q6v[:, 0:3, :])
            nc.gpsimd.tensor_copy(u5[:, 3:5, :], q6v[:, 0:2, :])
            w3 = wp.tile([128, 3, NJ], f32, tag="w3")
            for ci in range(3):
                nc.gpsimd.tensor_copy(w3[:, ci, :], npsegv[:, 3, :])
            rn23 = wp.tile([128, 3, NJ], f32, tag="rn23")
            for ci in range(3):
                nc.gpsimd.tensor_copy(rn23[:, ci, :], rn2[:])
            # s = v x u (+ dup), tv = v x s, m = w*s + tv, q6m = m*rn2
            t1 = wp.tile([128, 3, NJ], f32, tag="t1")
            t2 = wp.tile([128, 3, NJ], f32, tag="t2")
            s5 = wp.tile([128, 5, NJ], f32, tag="s5")
            nc.vector.tensor_mul(t1[:], v5[:, 1:4, :], u5[:, 2:5, :])
            nc.vector.tensor_mul(t2[:], v5[:, 2:5, :], u5[:, 1:4, :])
            nc.vector.tensor_sub(s5[:, 0:3, :], t1[:], t2[:])
            nc.gpsimd.tensor_copy(s5[:, 3:5, :], s5[:, 0:2, :])
            nc.vector.tensor_mul(t1[:], v5[:, 1:4, :], s5[:, 2:5, :])
            nc.vector.tensor_mul(t2[:], v5[:, 2:5, :], s5[:, 1:4, :])
            nc.vector.tensor_sub(t1[:], t1[:], t2[:])
            nc.vector.tensor_mul(t2[:], w3[:], s5[:, 0:3, :])
            nc.vector.tensor_add(t1[:], t1[:], t2[:])
            nc.vector.tensor_mul(q6v[:, 3:6, :], t1[:], rn23[:])

            # umT transposes: q6 j-block [128, 6] -> psum [6, 128] -> X1b
            for j in range(NJ):
                trj = pp_trp.tile([128, 128], f32, tag="trp", name="trj")
                nc.tensor.transpose(
                    trj[0:6, :], q6[:, 6 * j:6 * j + 6], ident[:]
                )
                nc.vector.tensor_copy(
                    X1b[0:6, 128 * j:128 * (j + 1)], trj[0:6, :]
                )

            # ---------------- W_eff part 1: pfcA rows ----------------
            psW0t = pp_pre.tile([128, 320], f32, tag="pre", name="psW0t")
            psW0 = psW0t[:, 0:256]
            for k in range(4):
                nc.tensor.matmul(
                    psW0, pfcAT[:, 128 * k:128 * (k + 1)], ow1[k],
                    start=(k == 0), stop=(k == 3),
                )

            # ---------------- timestep MLP ----------------
            h1p = pp_mlp.tile([32, C], f32, tag="mlp", name="h1p")
            nc.tensor.matmul(h1p[:], ones1[:], tb1r, start=True, stop=False)
            for k in range(4):
                nc.tensor.matmul(
                    h1p[:], embT[:, 32 * k:32 * (k + 1)], tw1p[k],
                    start=False, stop=(k == 3),
                )
            h1s = wp.tile([32, C], f32, tag="h1s")
            nc.scalar.activation(h1s[:], h1p[:], AF.Silu)
            # switch ACT to sqrt_and_others now (relu/copy/identity stay ok)
            nc.scalar.activation(dum3[:], h1s[0:1, 0:1], AF.Sqrt)
            h1sT = wp.tile([128, 4, 32], bf16, tag="h1sT")
            for k in range(4):
                tr = pp_trp.tile([128, 128], f32, tag="trp", name="tr1")
                nc.tensor.transpose(
                    tr[:, 0:32], h1s[:, 128 * k:128 * (k + 1)],
                    ident[0:32, 0:32]
                )
                nc.vector.tensor_copy(h1sT[:, k, :], tr[:, 0:32])
            t2p = pp_mlp.tile([32, C], f32, tag="mlp", name="t2p")
            nc.tensor.matmul(t2p[:], ones1[:], tb2r, start=True, stop=False)
            for k in range(4):
                nc.tensor.matmul(
                    t2p[:], h1sT[:, k, :], tw2[k],
                    start=False, stop=(k == 3),
                )
            temb2 = wp.tile([32, C], f32, tag="temb2")
            nc.scalar.activation(temb2[:], t2p[:], AF.Copy)
            for k in range(4):
                tr = pp_trp.tile([128, 128], f32, tag="trp", name="tr2")
                nc.tensor.transpose(
                    tr[:, 0:32], temb2[:, 128 * k:128 * (k + 1)],
                    ident[0:32, 0:32]
                )
                nc.vector.tensor_copy(Wb[:, k, 6:38], tr[:, 0:32])

            # ---------------- W_eff part 2 + copies ----------------
            nc.vector.tensor_add(
                Wa[:, :, 12:15], Wa[:, :, 12:15], pewT
            )
            psWat = pp_pre.tile([128, 320], f32, tag="pre", name="psWat")
            psWa = psWat[0:19, 0:256]
            for k in range(4):
                nc.tensor.matmul(
                    psWa, Wa[:, k, :], ow1[k],
                    start=(k == 0), stop=(k == 3),
                )
            psWbt = pp_pre.tile([128, 320], f32, tag="pre", name="psWbt")
            psWb = psWbt[0:38, 0:256]
            for k in range(4):
                nc.tensor.matmul(
                    psWb, Wb[:, k, :], ow1[k],
                    start=(k == 0), stop=(k == 3),
                )
            Weff0 = wp.tile([128, 256], bf16, tag="Weff0")
            nc.scalar.activation(Weff0[:], psW0, AF.Copy)
            Weffa = wp.tile([19, 256], bf16, tag="Weffa")
            nc.scalar.activation(Weffa[:], psWa, AF.Copy)
            Weffb = wp.tile([38, 256], bf16, tag="Weffb")
            nc.scalar.activation(Weffb[:], psWb, AF.Copy)

            # ---------------- h1T + BN1 ----------------
            stats1 = wp.tile([128, 24], f32, tag="stats1")
            relu1 = []
            bcols1 = wp.tile([128, 8], f32, tag="bcols1")
            scales1 = []
            psts = []
            for c in range(2):
                csl = slice(128 * c, 128 * (c + 1))
                pst = []
                for h in range(2):
                    sl = slice(320 * h, 320 * (h + 1))
                    ps = pp_head.tile([128, 320], f32, tag="hd",
                                      name=f"h1t{c}{h}")
                    nc.tensor.matmul(
                        ps[:], Weff0[:, csl], X0[:, sl],
                        start=True, stop=False,
                    )
                    nc.tensor.matmul(
                        ps[:], Weffa[:, csl], X1a[:, sl],
                        start=False, stop=False,
                    )
                    nc.tensor.matmul(
                        ps[:], Weffb[:, csl], X1b[:, sl],
                        start=False, stop=True,
                    )
                    nc.vector.bn_stats(
                        stats1[:, 12 * c + 6 * h:12 * c + 6 * h + 6], ps[:]
                    )
                    pst.append(ps)
                psts.append(pst)
                aggr = bcols1[:, 4 * c:4 * c + 2]
                nc.vector.bn_aggr(aggr, stats1[:, 12 * c:12 * c + 12])
                std = bcols1[:, 4 * c + 2:4 * c + 3]
                nc.scalar.activation(
                    std, aggr[:, 1:2], AF.Sqrt, bias=eps128[:, 0:1]
                )
                rstd = bcols1[:, 4 * c + 3:4 * c + 4]
                nc.vector.reciprocal(rstd, std)
                scale = wp.tile([128, 2], f32, tag=f"sc1{c}", name=f"sc1{c}")
                nc.vector.tensor_mul(scale[:, 0:1], rstd, bn1g[:, c:c + 1])
                nc.vector.tensor_mul(scale[:, 1:2], aggr[:, 0:1],
                                     scale[:, 0:1])
                nc.vector.tensor_sub(scale[:, 1:2], bn1b[:, c:c + 1],
                                     scale[:, 1:2])
                scales1.append(scale)
                r1 = wp.tile([128, S], bf16, tag=f"relu1{c}",
                             name=f"relu1{c}")
                relu1.append(r1)
            # applies, h-major so h2 matmuls start asap; h0 on ACT, h1 on DVE
            for h in range(2):
                sl = slice(320 * h, 320 * (h + 1))
                for c in range(2):
                    if h == 0:
                        nc.scalar.activation(
                            relu1[c][:, sl], psts[c][h][:], AF.Relu,
                            bias=scales1[c][:, 1:2], scale=scales1[c][:, 0:1],
                        )
                    else:
                        nc.vector.tensor_scalar(
                            relu1[c][:, sl], psts[c][h][:],
                            scales1[c][:, 0:1], scales1[c][:, 1:2],
                            op0=ALU.mult, op1=ALU.add,
                        )
                        nc.vector.tensor_scalar_max(
                            relu1[c][:, sl], relu1[c][:, sl], 0.0
                        )

            # ---------------- h2 + BN2 ----------------
            stats2 = wp.tile([128, 12], f32, tag="stats2")
            ps2t = []
            for h in range(2):
                sl = slice(320 * h, 320 * (h + 1))
                ps2 = pp_head.tile([128, 320], f32, tag="hd",
                                   name=f"h2t{h}")
                for cc in range(2):
                    nc.tensor.matmul(
                        ps2[:], ow2c[:, cc, :], relu1[cc][:, sl],
                        start=(cc == 0), stop=(cc == 1),
                    )
                nc.vector.bn_stats(stats2[:, 6 * h:6 * h + 6], ps2[:])
                ps2t.append(ps2)
            bcols2 = wp.tile([128, 4], f32, tag="bcols2")
            aggr2 = bcols2[:, 0:2]
            nc.vector.bn_aggr(aggr2, stats2[:])
            std2 = bcols2[:, 2:3]
            nc.scalar.activation(std2, aggr2[:, 1:2], AF.Sqrt,
                                 bias=eps128[:, 0:1])
            rstd2 = bcols2[:, 3:4]
            nc.vector.reciprocal(rstd2, std2)
            scale2 = wp.tile([128, 2], f32, tag="scale2")
            nc.vector.tensor_mul(scale2[:, 0:1], rstd2, bn2g[:])
            nc.vector.tensor_mul(scale2[:, 1:2], aggr2[:, 0:1],
                                 scale2[:, 0:1])
            nc.vector.tensor_sub(scale2[:, 1:2], bn2b[:], scale2[:, 1:2])
            relu2 = wp.tile([128, S], bf16, tag="relu2")
            for h in range(2):
                sl = slice(320 * h, 320 * (h + 1))
                if h == 0:
                    nc.scalar.activation(
                        relu2[:, sl], ps2t[h][:], AF.Relu,
                        bias=scale2[:, 1:2], scale=scale2[:, 0:1],
                    )
                else:
                    nc.vector.tensor_scalar(
                        relu2[:, sl], ps2t[h][:],
                        scale2[:, 0:1], scale2[:, 1:2],
                        op0=ALU.mult, op1=ALU.add,
                    )
                    nc.vector.tensor_scalar_max(
                        relu2[:, sl], relu2[:, sl], 0.0
                    )

            # ---------------- out ----------------
            out_sb = wp.tile([7, S], f32, tag="out_sb")
            for h in range(2):
                sl = slice(320 * h, 320 * (h + 1))
                ps3t = pp_head.tile([128, 320], f32, tag="hd",
                                    name=f"o{h}")
                ps3 = ps3t[0:7, :]
                nc.tensor.matmul(
                    ps3, ow3, relu2[:, sl], start=True, stop=True
                )
                nc.scalar.activation(
                    out_sb[:, sl], ps3, AF.Identity, bias=ob3c
                )
                nc.sync.dma_start(d_out[:, sl], out_sb[:, sl])

    nc.compile()
    return nc


def _in_maps(inp):
    GAs, GBs, fq, Bsel = _consts()
    f = np.float32
    bf = ml_dtypes.bfloat16

    def b(x):
        return np.ascontiguousarray(np.asarray(x, dtype=f).astype(bf))

    npar = np.ascontiguousarray(inp["noise_param"], dtype=f)
    pfc_w = np.asarray(inp["pfc_w"], dtype=f)
    pe_w = np.asarray(inp["pe_w"], dtype=f)

    # f32A [8, 1068]: xTf | GAs | GBs | fq(2 rows) | ones32(1 row)
    f32A = np.zeros((8, 1068), f)
    f32A[0:7, 0:640] = npar.T
    f32A[7, 0:640] = 1.0
    f32A[:, 640:768] = GAs
    f32A[:, 768:780] = GBs
    f32A[0:2, 780:1036] = fq
    f32A[0, 1036:1068] = 1.0

    # f32B [128, 43]: npseg | bn1g | bn1b | bn2g | bn2b | ob3
    f32B = np.zeros((128, 43), f)
    f32B[:, 0:35] = npar.reshape(NJ, 128, 7).transpose(1, 0, 2).reshape(
        128, NJ * 7)
    f32B[:, 35:37] = np.asarray(inp["bn1_g"], f).reshape(2, 128).T
    f32B[:, 37:39] = np.asarray(inp["bn1_b"], f).reshape(2, 128).T
    f32B[:, 39:40] = np.asarray(inp["bn2_g"], f).reshape(128, 1)
    f32B[:, 40:41] = np.asarray(inp["bn2_b"], f).reshape(128, 1)
    f32B[0:7, 41] = np.asarray(inp["o_b3"], f)

    # bfS [1, 1024]: t_b1 | t_b2
    bfS = np.zeros((1, 1024), f)
    bfS[0, 0:512] = np.asarray(inp["t_b1"], f)
    bfS[0, 512:1024] = np.asarray(inp["t_b2"], f)

    # W1 [128, 3824]: tw1p(2048) | pfcAT(512) | ow1(1024) | Wa(76) |
    #                 Wb(152) | pewT(12)
    tw1 = np.asarray(inp["t_w1"], dtype=f)
    perm = np.concatenate([
        np.arange(0, 128), np.arange(256, 384),
        np.arange(128, 256), np.arange(384, 512),
    ])
    tw1p = tw1[perm]
    ow1 = np.asarray(inp["o_w1"], dtype=f)
    A = pfc_w[7:135]
    W1 = np.zeros((128, 3824), f)
    for k in range(4):
        ch = slice(128 * k, 128 * (k + 1))
        W1[:, 512 * k:512 * (k + 1)] = tw1p[ch]
        W1[:, 2048 + 128 * k:2048 + 128 * (k + 1)] = A[:, ch].T
        W1[:, 2560 + 256 * k:2560 + 256 * (k + 1)] = ow1[ch]
        W1[:, 3584 + 19 * k:3584 + 19 * k + 12] = pfc_w[135:147, ch].T
        W1[:, 3584 + 19 * k + 12:3584 + 19 * k + 19] = pfc_w[0:7, ch].T
        W1[:, 3660 + 38 * k:3660 + 38 * k + 3] = pe_w[:, ch].T / PPP
        W1[:, 3660 + 38 * k + 3:3660 + 38 * k + 6] = (
            pe_w[:, ch].T * (2.0 / PPP))
        W1[:, 3812 + 3 * k:3812 + 3 * (k + 1)] = pe_w[:, ch].T

    # W2 [128, 2311]: tw2(2048) | ow2c(256) | ow3(7)
    tw2 = np.asarray(inp["t_w2"], dtype=f)
    ow2 = np.asarray(inp["o_w2"], dtype=f)
    W2 = np.zeros((128, 2311), f)
    for k in range(4):
        W2[:, 512 * k:512 * (k + 1)] = tw2[128 * k:128 * (k + 1)]
    for k in range(2):
        W2[:, 2048 + 128 * k:2048 + 128 * (k + 1)] = (
            ow2[128 * k:128 * (k + 1)])
    W2[:, 2304:2311] = np.asarray(inp["o_w3"], dtype=f)

    base = {
        "f32A": f32A,
        "f32B": f32B,
        "ts": np.ascontiguousarray(
            np.asarray(inp["timesteps"]).reshape(1, BO).astype(np.int32)
        ),
        "bfS": b(bfS),
        "W1": b(W1),
        "W2": b(W2),
        "pc": b(
            np.asarray(inp["part_pcs"], dtype=f)
            .reshape(S, PPP, 3).transpose(0, 2, 1).reshape(S, PPP * 3)
        ),
        "xTb": b(npar.T),
        "Bsel": b(Bsel),
    }
    return [dict(base) for _ in range(NCORES)]


def _ensure_axon_hooks():
    # The agent image's `antenv` lacks `axon_hooks`; bass_utils imports it
    # unconditionally when tracing under axon. Provide it (and register the
    # real NTFF hook from trn_boot) so trace=True / BASS_TRACE=1 work.
    try:
        import antenv.axon_hooks  # noqa: F401
        return
    except ImportError:
        pass
    import sys
    import types

    mod = types.ModuleType("antenv.axon_hooks")
    _hook = [None]
    mod.set_axon_ntff_profile_hook = lambda h: _hook.__setitem__(0, h)
    mod.get_axon_ntff_profile_hook = lambda: _hook[0]
    sys.modules["antenv.axon_hooks"] = mod
    try:
        import antenv

        antenv.axon_hooks = mod
    except ImportError:
        pass
    try:
        from trn_agent_boot.trn_boot import _ntff_profile_via_ctypes

        mod.set_axon_ntff_profile_hook(
            _ntff_profile_via_ctypes("/opt/axon/libaxon_pjrt.so")
        )
    except Exception:
        pass


def _run(inputs, trace=False):
    _ensure_axon_hooks()
    from concourse.bass_utils import run_bass_kernel_spmd

    if "nc" not in _CACHE:
        _CACHE["nc"] = _build_nc()
    res = run_bass_kernel_spmd(
        _CACHE["nc"], _in_maps(inputs), list(range(NCORES)), trace=trace
    )
    out = np.ascontiguousarray(
        np.asarray(res.results[0]["outT"]).T.astype(np.float32)
    )
    return out, res


def kernel(**inputs):
    inp = {k: np.asarray(v) for k, v in inputs.items()}
    out, _ = _run(inp)
    return out


# revision 17
# speedup vs baseline: 1.0321x; 1.0321x over previous
"""Trainium2 Bass kernel for nn_DiffModel_53764400611855.

The 160000-point stream collapses algebraically to per-segment coordinate
sums u[s] (segment_sum and quat rotation are linear in the points), and the
batchnorm layers cancel every bias that is constant across the 640-segment
batch (pe_b, pfc_b, o_b1, o_b2).  What remains is:

  h1T = (W_all @ o_w1)^T @ X_all          with
  W_all rows / X_all rows:
     pfcA   (128) <->  nerfA  = sin(2pi * reduce(GA' x + bA'))   [128,640]
     pfcBs   (12) <->  nerfBs = sin(...)                          [12,640]
     pfcBi+pe_w(7)<->  xT     = noise_param^T                      [7,640]
     pe_w     (3) <->  uT     = per-seg point sums / 250           [3,640]
     2*pe_w/250(3)<->  mT     = (w*(v x u) + v x (v x u)) / |q|^2  [3,640]
     temb2   (32) <->  Bsel   = kron(I32, 1_20)                   [32,640]
  then bn+relu -> @o_w2 -> bn+relu -> @o_w3 + b3.

All matmuls run in bf16 (fp32 PSUM accumulate) except the trig-argument
matmuls, which stay fp32 for phase accuracy.  sin() uses a 3-op range
reduction (f32->i32 cast rounds to nearest on this HW) + one ACT Sin with
scale=2pi.  BatchNorm moments come from bn_stats/bn_aggr; the scale, shift,
relu and bf16 cast fuse into one ACT per tile.  Only two ACT table sets are
used (silu_and_others, sqrt_and_others).

All 8 cores run the same replicated program (no collectives); core 0's
output is returned.  Hardcodes the fixed input structure: contiguous
segments of 250 points, batch_length == 250.
"""

import numpy as np
import ml_dtypes

NCORES = 8
S, C, PPP, BO = 640, 512, 250, 32
NJ = S // 128               # seg-major blocks = 5
PI = float(np.pi)
TWO_PI = float(2.0 * np.pi)
INV2PI = float(1.0 / (2.0 * np.pi))

_CACHE = {}


def _consts():
    f = np.float32
    # nerf A block: sc-flat cols 0..127 (bands 0..9 partial), with /2pi
    # prescale and bias row (0.25 turn for cos entries)
    GAs = np.zeros((8, 128), f)
    for i in range(128):
        fb, k = i // 14, i % 14
        GAs[k % 7, i] = (2.0 ** fb) * INV2PI
        GAs[7, i] = 0.25 if k >= 7 else 0.0
    # B block: sc-flat cols 128..139 (band 9, k=2..13)
    GBs = np.zeros((8, 12), f)
    for j in range(12):
        k = 2 + j
        GBs[k % 7, j] = (2.0 ** 9) * INV2PI
        GBs[7, j] = 0.25 if k >= 7 else 0.0
    freqs = np.exp(
        -np.log(10000.0) * np.arange(256, dtype=f) / 256.0
    ).astype(f)
    fq = np.zeros((2, 256), f)
    fq[0] = freqs * INV2PI
    fq[1] = 0.25
    Bsel = np.kron(np.eye(BO, dtype=f), np.ones((1, 20), f))
    return GAs, GBs, fq, np.ascontiguousarray(Bsel)


def _build_nc():
    import concourse.mybir as mybir
    import concourse.tile as tile
    from concourse import bacc, masks

    f32, i32, bf16 = mybir.dt.float32, mybir.dt.int32, mybir.dt.bfloat16
    AF = mybir.ActivationFunctionType
    ALU = mybir.AluOpType
    AX = mybir.AxisListType

    nc = bacc.Bacc(None, num_devices=NCORES)

    def din(name, shape, dt=f32):
        return nc.dram_tensor(name, shape, dt, kind="ExternalInput")

    # consolidated input blobs (few big DMAs; see _in_maps for layouts)
    d_f32A = din("f32A", [8, 1068])
    d_f32B = din("f32B", [128, 43])
    d_ts = din("ts", [1, BO], i32)
    d_bfS = din("bfS", [1, 1024], bf16)
    d_W1 = din("W1", [128, 3824], bf16)
    d_W2 = din("W2", [128, 2311], bf16)
    d_pc = din("pc", [128, NJ * PPP * 3], bf16)
    d_xTb = din("xTb", [7, S], bf16)
    d_Bsel = din("Bsel", [BO, S], bf16)
    d_out = nc.dram_tensor("outT", [7, S], f32, kind="ExternalOutput")

    with tile.TileContext(nc) as tc:
        with (
            tc.tile_pool(name="const", bufs=1) as cp,
            tc.tile_pool(name="work", bufs=1) as wp,
            tc.tile_pool(name="ps_pre", bufs=2, space="PSUM") as pp_pre,
            tc.tile_pool(name="ps_mlp", bufs=1, space="PSUM") as pp_mlp,
            tc.tile_pool(name="ps_trp", bufs=1, space="PSUM") as pp_trp,
            tc.tile_pool(name="ps_head", bufs=4, space="PSUM") as pp_head,
        ):
            # ---------------- DMAs ----------------
            # sync ring: small f32 first, then pc j0-1, then weight blobs
            f32A = cp.tile([8, 1068], f32, tag="f32A")
            nc.sync.dma_start(f32A[:], d_f32A[:])
            ts_i = cp.tile([1, BO], i32, tag="ts_i")
            nc.sync.dma_start(ts_i[:], d_ts[:])
            f32B = cp.tile([128, 43], f32, tag="f32B")
            nc.sync.dma_start(f32B[:], d_f32B[:])
            bfS = cp.tile([1, 1024], bf16, tag="bfS")
            nc.sync.dma_start(bfS[:], d_bfS[:])
            pcb = wp.tile([128, NJ, PPP * 3], bf16, tag="pcb")
            pc_r = d_pc.rearrange("p (j k) -> p j k", j=NJ)
            nc.sync.dma_start(pcb[:, 0:2, :], pc_r[:, 0:2, :])
            W1 = cp.tile([128, 3824], bf16, tag="W1")
            nc.sync.dma_start(W1[:], d_W1[:])
            W2 = cp.tile([128, 2311], bf16, tag="W2")
            nc.sync.dma_start(W2[:], d_W2[:])
            # scalar ring (2nd HWDGE): pc j2-4 + X-side inputs
            nc.scalar.dma_start(pcb[:, 2:5, :], pc_r[:, 2:5, :])
            X1a = wp.tile([19, S], bf16, tag="X1a")
            nc.scalar.dma_start(X1a[12:19, :], d_xTb[:])
            X1b = wp.tile([38, S], bf16, tag="X1b")
            nc.scalar.dma_start(X1b[6:38, :], d_Bsel[:])

            # views into the blobs
            xTf = f32A[:, 0:640]
            GAs = f32A[:, 640:768]
            GBs = f32A[:, 768:780]
            fq = f32A[0:2, 780:1036]
            npseg = f32B[:, 0:35]
            bn1g = f32B[:, 35:37]
            bn1b = f32B[:, 37:39]
            bn2g = f32B[:, 39:40]
            bn2b = f32B[:, 40:41]
            ob3c = f32B[0:7, 41:42]
            tb1r = bfS[0:1, 0:512]
            tb2r = bfS[0:1, 512:1024]
            tw1p = [W1[:, 512 * k:512 * (k + 1)] for k in range(4)]
            pfcAT = W1[:, 2048:2560]
            ow1 = [W1[:, 2560 + 256 * k:2560 + 256 * (k + 1)]
                   for k in range(4)]
            Wa = W1[:, 3584:3660].rearrange("p (k r) -> p k r", r=19)
            Wb = W1[:, 3660:3812].rearrange("p (k r) -> p k r", r=38)
            pewT = W1[:, 3812:3824].rearrange("p (k r) -> p k r", r=3)
            tw2 = [W2[:, 512 * k:512 * (k + 1)] for k in range(4)]
            ow2c = W2[:, 2048:2304].rearrange("p (k n) -> p k n", n=128)
            ow3 = W2[:, 2304:2311]

            ident = cp.tile([128, 128], f32, tag="ident")
            masks.make_identity(nc, ident[:])
            ones1 = cp.tile([1, BO], bf16, tag="ones1")
            nc.gpsimd.memset(ones1[:], 1.0)
            dum = cp.tile([1, 1], f32, tag="dum")
            nc.gpsimd.memset(dum[:], 1.0)
            dum2 = cp.tile([1, 1], f32, tag="dum2")
            dum3 = cp.tile([1, 1], f32, tag="dum3")
            eps128 = cp.tile([128, 1], f32, tag="eps128")
            nc.gpsimd.memset(eps128[:], 1e-5)
            # first ACT op -> loads silu_and_others (sin+silu+copy+relu)
            nc.scalar.activation(dum2[:], dum[:], AF.Silu)

            tm2 = wp.tile([2, BO], f32, tag="tm2")
            nc.vector.tensor_copy(tm2[0:1, :], ts_i[:])
            nc.sync.dma_start(tm2[1:2, :], d_f32A[0:1, 1036:1068])

            # q6 cols per j: u(3) m(3); u by reduces, m by quat
            q6 = wp.tile([128, NJ * 6], f32, tag="q6")

            def reduce_j(j):
                nc.vector.tensor_reduce(
                    q6[:, 6 * j:6 * j + 3],
                    pcb[:, j, :].rearrange("p (c k) -> p c k", c=3),
                    axis=AX.X, op=ALU.add,
                )

            MAGIC = float(1.5 * 2.0 ** 23)

            def frac_chain(ps_ap, P, W, tag, rr_view):
                # rr = ps - round(ps) via magic-number round (RNE)
                rnd_ = wp.tile([P, W], f32, tag=f"{tag}n", name=f"{tag}n")
                nc.vector.tensor_scalar(
                    rnd_[:], ps_ap, MAGIC, -MAGIC, op0=ALU.add, op1=ALU.add
                )
                nc.vector.tensor_tensor(
                    rr_view, ps_ap, rnd_[:], op=ALU.subtract
                )

            # ---- reduce j2 first (scalar-ring pc lands first) ----
            reduce_j(2)

            # ---- argt: [cos0 | sin0 | cos1 | sin1] blocks of 32 ----
            argt = pp_trp.tile([128, 128], f32, tag="trp", name="argt")
            for r in range(2):
                fsl = slice(128 * r, 128 * (r + 1))
                nc.tensor.matmul(
                    argt[:, 64 * r:64 * r + 32], fq[:, fsl], tm2[:],
                    start=True, stop=True,
                )
                nc.tensor.matmul(
                    argt[:, 64 * r + 32:64 * r + 64], fq[0:1, fsl],
                    tm2[0:1, :], start=True, stop=True,
                )
            rrT = wp.tile([128, 128], f32, tag="rrT")
            frac_chain(argt[:], 128, 128, "at", rrT[:])
            embT = wp.tile([128, 128], bf16, tag="embT")
            nc.scalar.activation(embT[:], rrT[:], AF.Sin, scale=TWO_PI)

            reduce_j(3)
            reduce_j(4)

            # ---- nerf A args + trig ----
            X0 = wp.tile([128, S], bf16, tag="X0")
            rrA = wp.tile([128, S], f32, tag="rrA")
            for h in range(2):
                sl = slice(320 * h, 320 * (h + 1))
                psA = pp_pre.tile([128, 320], f32, tag="pre", name="psA")
                nc.tensor.matmul(
                    psA[:], GAs, xTf[:, sl], start=True, stop=True
                )
                frac_chain(psA[:], 128, 320, f"nA{h}", rrA[:, sl])
            nc.scalar.activation(X0[:], rrA[:], AF.Sin, scale=TWO_PI)

            reduce_j(0)
            reduce_j(1)

            # ---- nerf B args + trig ----
            rrB = wp.tile([12, S], f32, tag="rrB")
            for h in range(2):
                sl = slice(320 * h, 320 * (h + 1))
                psB = pp_pre.tile([128, 320], f32, tag="pre", name="psB")
                nc.tensor.matmul(
                    psB[0:12, :], GBs, xTf[:, sl], start=True, stop=True
                )
                frac_chain(psB[0:12, :], 12, 320, f"nB{h}", rrB[:, sl])
            nc.scalar.activation(X1a[0:12, :], rrB[:], AF.Sin, scale=TWO_PI)

            # ---------------- W_eff part 1 + timestep MLP (PE) ----------
            psW0t = pp_pre.tile([128, 320], f32, tag="pre", name="psW0t")
            psW0 = psW0t[:, 0:256]
            for k in range(4):
                nc.tensor.matmul(
                    psW0, pfcAT[:, 128 * k:128 * (k + 1)], ow1[k],
                    start=(k == 0), stop=(k == 3),
                )
            h1p = pp_mlp.tile([32, C], f32, tag="mlp", name="h1p")
            nc.tensor.matmul(h1p[:], ones1[:], tb1r, start=True, stop=False)
            for k in range(4):
                nc.tensor.matmul(
                    h1p[:], embT[:, 32 * k:32 * (k + 1)], tw1p[k],
                    start=False, stop=(k == 3),
                )
            h1s = wp.tile([32, C], f32, tag="h1s")
            nc.scalar.activation(h1s[:], h1p[:], AF.Silu)
            # switch ACT to sqrt_and_others now (relu/copy/identity stay ok)
            nc.scalar.activation(dum3[:], h1s[0:1, 0:1], AF.Sqrt)
            h1sT = wp.tile([128, 4, 32], bf16, tag="h1sT")
            for k in range(4):
                tr = pp_trp.tile([128, 128], f32, tag="trp", name="tr1")
                nc.tensor.transpose(
                    tr[:, 0:32], h1s[:, 128 * k:128 * (k + 1)],
                    ident[0:32, 0:32]
                )
                nc.vector.tensor_copy(h1sT[:, k, :], tr[:, 0:32])
            t2p = pp_mlp.tile([32, C], f32, tag="mlp", name="t2p")
            nc.tensor.matmul(t2p[:], ones1[:], tb2r, start=True, stop=False)
            for k in range(4):
                nc.tensor.matmul(
                    t2p[:], h1sT[:, k, :], tw2[k],
                    start=False, stop=(k == 3),
                )
            temb2 = wp.tile([32, C], f32, tag="temb2")
            nc.scalar.activation(temb2[:], t2p[:], AF.Copy)
            for k in range(4):
                tr = pp_trp.tile([128, 128], f32, tag="trp", name="tr2")
                nc.tensor.transpose(
                    tr[:, 0:32], temb2[:, 128 * k:128 * (k + 1)],
                    ident[0:32, 0:32]
                )
                nc.vector.tensor_copy(Wb[:, k, 6:38], tr[:, 0:32])

            # ---------------- W_eff part 2 ----------------
            nc.vector.tensor_add(
                Wa[:, :, 12:15], Wa[:, :, 12:15], pewT
            )
            psWat = pp_pre.tile([128, 320], f32, tag="pre", name="psWat")
            psWa = psWat[0:19, 0:256]
            for k in range(4):
                nc.tensor.matmul(
                    psWa, Wa[:, k, :], ow1[k],
                    start=(k == 0), stop=(k == 3),
                )
            psWbt = pp_pre.tile([128, 320], f32, tag="pre", name="psWbt")
            psWb = psWbt[0:38, 0:256]
            for k in range(4):
                nc.tensor.matmul(
                    psWb, Wb[:, k, :], ow1[k],
                    start=(k == 0), stop=(k == 3),
                )
            Weff0 = wp.tile([128, 256], bf16, tag="Weff0")
            nc.scalar.activation(Weff0[:], psW0, AF.Copy)
            Weffa = wp.tile([19, 256], bf16, tag="Weffa")
            nc.scalar.activation(Weffa[:], psWa, AF.Copy)
            Weffb = wp.tile([38, 256], bf16, tag="Weffb")
            nc.scalar.activation(Weffb[:], psWb, AF.Copy)

            # ---------------- quaternions (comp-major packed) -----------
            npsegv = npseg.rearrange("p (j c) -> p c j", c=7)
            q6v = q6[:, :].rearrange("p (j c) -> p c j", c=6)
            sq = wp.tile([128, NJ * 4], f32, tag="sq")
            sq_v = sq[:, :].rearrange("p (j c) -> p j c", c=4)
            nc.vector.tensor_mul(
                sq_v, npseg.rearrange("p (j c) -> p j c", c=7)[:, :, 3:7],
                npseg.rearrange("p (j c) -> p j c", c=7)[:, :, 3:7],
            )
            n2 = wp.tile([128, NJ], f32, tag="n2")
            nc.vector.tensor_reduce(n2[:], sq_v, axis=AX.X, op=ALU.add)
            rn2 = wp.tile([128, NJ], f32, tag="rn2")
            nc.vector.reciprocal(rn2[:], n2[:])
            v5 = wp.tile([128, 5, NJ], f32, tag="v5")
            nc.gpsimd.tensor_copy(v5[:, 0:3, :], npsegv[:, 4:7, :])
            nc.gpsimd.tensor_copy(v5[:, 3:5, :], npsegv[:, 4:6, :])
            u5 = wp.tile([128, 5, NJ], f32, tag="u5")
            nc.gpsimd.tensor_copy(u5[:, 0:3, :], q6v[:, 0:3, :])
            nc.gpsimd.tensor_copy(u5[:, 3:5, :], q6v[:, 0:2, :])
            w3 = wp.tile([128, 3, NJ], f32, tag="w3")
            for ci in range(3):
                nc.gpsimd.tensor_copy(w3[:, ci, :], npsegv[:, 3, :])
            rn23 = wp.tile([128, 3, NJ], f32, tag="rn23")
            for ci in range(3):
                nc.gpsimd.tensor_copy(rn23[:, ci, :], rn2[:])
            t1 = wp.tile([128, 3, NJ], f32, tag="t1")
            t2 = wp.tile([128, 3, NJ], f32, tag="t2")
            s5 = wp.tile([128, 5, NJ], f32, tag="s5")
            nc.vector.tensor_mul(t1[:], v5[:, 1:4, :], u5[:, 2:5, :])
            nc.vector.tensor_mul(t2[:], v5[:, 2:5, :], u5[:, 1:4, :])
            nc.vector.tensor_sub(s5[:, 0:3, :], t1[:], t2[:])
            nc.gpsimd.tensor_copy(s5[:, 3:5, :], s5[:, 0:2, :])
            nc.vector.tensor_mul(t1[:], v5[:, 1:4, :], s5[:, 2:5, :])
            nc.vector.tensor_mul(t2[:], v5[:, 2:5, :], s5[:, 1:4, :])
            nc.vector.tensor_sub(t1[:], t1[:], t2[:])
            nc.vector.tensor_mul(t2[:], w3[:], s5[:, 0:3, :])
            nc.vector.tensor_add(t1[:], t1[:], t2[:])
            nc.vector.tensor_mul(q6v[:, 3:6, :], t1[:], rn23[:])

            # umT transposes (PE emitted after W_eff) -> X1b rows 0:6
            for j in range(NJ):
                trj = pp_trp.tile([128, 128], f32, tag="trp", name="trj")
                nc.tensor.transpose(
                    trj[0:6, :], q6[:, 6 * j:6 * j + 6], ident[:]
                )
                nc.vector.tensor_copy(
                    X1b[0:6, 128 * j:128 * (j + 1)], trj[0:6, :]
                )

            # ---------------- h1T + BN1 ----------------
            stats1 = wp.tile([128, 24], f32, tag="stats1")
            relu1 = []
            bcols1 = wp.tile([128, 8], f32, tag="bcols1")
            scales1 = []
            psts = []
            for c in range(2):
                csl = slice(128 * c, 128 * (c + 1))
                pst = []
                for h in range(2):
                    sl = slice(320 * h, 320 * (h + 1))
                    ps = pp_head.tile([128, 320], f32, tag="hd",
                                      name=f"h1t{c}{h}")
                    nc.tensor.matmul(
                        ps[:], Weff0[:, csl], X0[:, sl],
                        start=True, stop=False,
                    )
                    nc.tensor.matmul(
                        ps[:], Weffa[:, csl], X1a[:, sl],
                        start=False, stop=False,
                    )
                    nc.tensor.matmul(
                        ps[:], Weffb[:, csl], X1b[:, sl],
                        start=False, stop=True,
                    )
                    nc.vector.bn_stats(
                        stats1[:, 12 * c + 6 * h:12 * c + 6 * h + 6], ps[:]
                    )
                    pst.append(ps)
                psts.append(pst)
                aggr = bcols1[:, 4 * c:4 * c + 2]
                nc.vector.bn_aggr(aggr, stats1[:, 12 * c:12 * c + 12])
                std = bcols1[:, 4 * c + 2:4 * c + 3]
                nc.scalar.activation(
                    std, aggr[:, 1:2], AF.Sqrt, bias=eps128[:, 0:1]
                )
                rstd = bcols1[:, 4 * c + 3:4 * c + 4]
                nc.vector.reciprocal(rstd, std)
                scale = wp.tile([128, 2], f32, tag=f"sc1{c}", name=f"sc1{c}")
                nc.vector.tensor_mul(scale[:, 0:1], rstd, bn1g[:, c:c + 1])
                nc.vector.tensor_mul(scale[:, 1:2], aggr[:, 0:1],
                                     scale[:, 0:1])
                nc.vector.tensor_sub(scale[:, 1:2], bn1b[:, c:c + 1],
                                     scale[:, 1:2])
                scales1.append(scale)
                r1 = wp.tile([128, S], bf16, tag=f"relu1{c}",
                             name=f"relu1{c}")
                relu1.append(r1)
            # applies, h-major so h2 matmuls start asap; h0 on ACT, h1 on DVE
            for h in range(2):
                sl = slice(320 * h, 320 * (h + 1))
                for c in range(2):
                    if h == 0:
                        nc.scalar.activation(
                            relu1[c][:, sl], psts[c][h][:], AF.Relu,
                            bias=scales1[c][:, 1:2], scale=scales1[c][:, 0:1],
                        )
                    else:
                        nc.vector.tensor_scalar(
                            relu1[c][:, sl], psts[c][h][:],
                            scales1[c][:, 0:1], scales1[c][:, 1:2],
                            op0=ALU.mult, op1=ALU.add,
                        )
                        nc.vector.tensor_scalar_max(
                            relu1[c][:, sl], relu1[c][:, sl], 0.0
                        )

            # ---------------- h2 + BN2 ----------------
            stats2 = wp.tile([128, 12], f32, tag="stats2")
            ps2t = []
            for h in range(2):
                sl = slice(320 * h, 320 * (h + 1))
                ps2 = pp_head.tile([128, 320], f32, tag="hd",
                                   name=f"h2t{h}")
                for cc in range(2):
                    nc.tensor.matmul(
                        ps2[:], ow2c[:, cc, :], relu1[cc][:, sl],
                        start=(cc == 0), stop=(cc == 1),
                    )
                nc.vector.bn_stats(stats2[:, 6 * h:6 * h + 6], ps2[:])
                ps2t.append(ps2)
            bcols2 = wp.tile([128, 4], f32, tag="bcols2")
            aggr2 = bcols2[:, 0:2]
            nc.vector.bn_aggr(aggr2, stats2[:])
            std2 = bcols2[:, 2:3]
            nc.scalar.activation(std2, aggr2[:, 1:2], AF.Sqrt,
                                 bias=eps128[:, 0:1])
            rstd2 = bcols2[:, 3:4]
            nc.vector.reciprocal(rstd2, std2)
            scale2 = wp.tile([128, 2], f32, tag="scale2")
            nc.vector.tensor_mul(scale2[:, 0:1], rstd2, bn2g[:])
            nc.vector.tensor_mul(scale2[:, 1:2], aggr2[:, 0:1],
                                 scale2[:, 0:1])
            nc.vector.tensor_sub(scale2[:, 1:2], bn2b[:], scale2[:, 1:2])
            relu2 = wp.tile([128, S], bf16, tag="relu2")
            for h in range(2):
                sl = slice(320 * h, 320 * (h + 1))
                if h == 0:
                    nc.scalar.activation(
                        relu2[:, sl], ps2t[h][:], AF.Relu,
                        bias=scale2[:, 1:2], scale=scale2[:, 0:1],
                    )
                else:
                    nc.vector.tensor_scalar(
                        relu2[:, sl], ps2t[h][:],
                        scale2[:, 0:1], scale2[:, 1:2],
                        op0=ALU.mult, op1=ALU.add,
                    )
                    nc.vector.tensor_scalar_max(
                        relu2[:, sl], relu2[:, sl], 0.0
                    )

            # ---------------- out ----------------
            out_sb = wp.tile([7, S], f32, tag="out_sb")
            for h in range(2):
                sl = slice(320 * h, 320 * (h + 1))
                ps3t = pp_head.tile([128, 320], f32, tag="hd",
                                    name=f"o{h}")
                ps3 = ps3t[0:7, :]
                nc.tensor.matmul(
                    ps3, ow3, relu2[:, sl], start=True, stop=True
                )
                nc.scalar.activation(
                    out_sb[:, sl], ps3, AF.Identity, bias=ob3c
                )
                nc.sync.dma_start(d_out[:, sl], out_sb[:, sl])

    nc.compile()
    return nc


def _in_maps(inp):
    GAs, GBs, fq, Bsel = _consts()
    f = np.float32
    bf = ml_dtypes.bfloat16

    def b(x):
        return np.ascontiguousarray(np.asarray(x, dtype=f).astype(bf))

    npar = np.ascontiguousarray(inp["noise_param"], dtype=f)
    pfc_w = np.asarray(inp["pfc_w"], dtype=f)
    pe_w = np.asarray(inp["pe_w"], dtype=f)

    # f32A [8, 1068]: xTf | GAs | GBs | fq(2 rows) | ones32(1 row)
    f32A = np.zeros((8, 1068), f)
    f32A[0:7, 0:640] = npar.T
    f32A[7, 0:640] = 1.0
    f32A[:, 640:768] = GAs
    f32A[:, 768:780] = GBs
    f32A[0:2, 780:1036] = fq
    f32A[0, 1036:1068] = 1.0

    # f32B [128, 43]: npseg | bn1g | bn1b | bn2g | bn2b | ob3
    f32B = np.zeros((128, 43), f)
    f32B[:, 0:35] = npar.reshape(NJ, 128, 7).transpose(1, 0, 2).reshape(
        128, NJ * 7)
    f32B[:, 35:37] = np.asarray(inp["bn1_g"], f).reshape(2, 128).T
    f32B[:, 37:39] = np.asarray(inp["bn1_b"], f).reshape(2, 128).T
    f32B[:, 39:40] = np.asarray(inp["bn2_g"], f).reshape(128, 1)
    f32B[:, 40:41] = np.asarray(inp["bn2_b"], f).reshape(128, 1)
    f32B[0:7, 41] = np.asarray(inp["o_b3"], f)

    # bfS [1, 1024]: t_b1 | t_b2
    bfS = np.zeros((1, 1024), f)
    bfS[0, 0:512] = np.asarray(inp["t_b1"], f)
    bfS[0, 512:1024] = np.asarray(inp["t_b2"], f)

    # W1 [128, 3824]: tw1p(2048) | pfcAT(512) | ow1(1024) | Wa(76) |
    #                 Wb(152) | pewT(12)
    tw1 = np.asarray(inp["t_w1"], dtype=f)
    perm = np.concatenate([
        np.arange(0, 128), np.arange(256, 384),
        np.arange(128, 256), np.arange(384, 512),
    ])
    tw1p = tw1[perm]
    ow1 = np.asarray(inp["o_w1"], dtype=f)
    A = pfc_w[7:135]
    W1 = np.zeros((128, 3824), f)
    for k in range(4):
        ch = slice(128 * k, 128 * (k + 1))
        W1[:, 512 * k:512 * (k + 1)] = tw1p[ch]
        W1[:, 2048 + 128 * k:2048 + 128 * (k + 1)] = A[:, ch].T
        W1[:, 2560 + 256 * k:2560 + 256 * (k + 1)] = ow1[ch]
        W1[:, 3584 + 19 * k:3584 + 19 * k + 12] = pfc_w[135:147, ch].T
        W1[:, 3584 + 19 * k + 12:3584 + 19 * k + 19] = pfc_w[0:7, ch].T
        W1[:, 3660 + 38 * k:3660 + 38 * k + 3] = pe_w[:, ch].T / PPP
        W1[:, 3660 + 38 * k + 3:3660 + 38 * k + 6] = (
            pe_w[:, ch].T * (2.0 / PPP))
        W1[:, 3812 + 3 * k:3812 + 3 * (k + 1)] = pe_w[:, ch].T

    # W2 [128, 2311]: tw2(2048) | ow2c(256) | ow3(7)
    tw2 = np.asarray(inp["t_w2"], dtype=f)
    ow2 = np.asarray(inp["o_w2"], dtype=f)
    W2 = np.zeros((128, 2311), f)
    for k in range(4):
        W2[:, 512 * k:512 * (k + 1)] = tw2[128 * k:128 * (k + 1)]
    for k in range(2):
        W2[:, 2048 + 128 * k:2048 + 128 * (k + 1)] = (
            ow2[128 * k:128 * (k + 1)])
    W2[:, 2304:2311] = np.asarray(inp["o_w3"], dtype=f)

    base = {
        "f32A": f32A,
        "f32B": f32B,
        "ts": np.ascontiguousarray(
            np.asarray(inp["timesteps"]).reshape(1, BO).astype(np.int32)
        ),
        "bfS": b(bfS),
        "W1": b(W1),
        "W2": b(W2),
        "pc": b(
            np.asarray(inp["part_pcs"], dtype=f)
            .reshape(NJ, 128, PPP, 3).transpose(1, 0, 3, 2)
            .reshape(128, NJ * 3 * PPP)
        ),
        "xTb": b(npar.T),
        "Bsel": b(Bsel),
    }
    return [dict(base) for _ in range(NCORES)]


def _ensure_axon_hooks():
    # The agent image's `antenv` lacks `axon_hooks`; bass_utils imports it
    # unconditionally when tracing under axon. Provide it (and register the
    # real NTFF hook from trn_boot) so trace=True / BASS_TRACE=1 work.
    try:
        import antenv.axon_hooks  # noqa: F401
        return
    except ImportError:
        pass
    import sys
    import types

    mod = types.ModuleType("antenv.axon_hooks")
    _hook = [None]
    mod.set_axon_ntff_profile_hook = lambda h: _hook.__setitem__(0, h)
    mod.get_axon_ntff_profile_hook = lambda: _hook[0]
    sys.modules["antenv.axon_hooks"] = mod
    try:
        import antenv

        antenv.axon_hooks = mod
    except ImportError:
        pass
    try:
        from trn_agent_boot.trn_boot import _ntff_profile_via_ctypes

        mod.set_axon_ntff_profile_hook(
            _ntff_profile_via_ctypes("/opt/axon/libaxon_pjrt.so")
        )
    except Exception:
        pass


def _run(inputs, trace=False):
    _ensure_axon_hooks()
    from concourse.bass_utils import run_bass_kernel_spmd

    if "nc" not in _CACHE:
        _CACHE["nc"] = _build_nc()
    res = run_bass_kernel_spmd(
        _CACHE["nc"], _in_maps(inputs), list(range(NCORES)), trace=trace
    )
    out = np.ascontiguousarray(
        np.asarray(res.results[0]["outT"]).T.astype(np.float32)
    )
    return out, res


def kernel(**inputs):
    inp = {k: np.asarray(v) for k, v in inputs.items()}
    out, _ = _run(inp)
    return out
